# revision 1
# baseline (speedup 1.0000x reference)
"""Trainium2 Bass kernel for GroupedQuerySelfAttention.

Problem: B=2, N=2048, D=2048, H=8 kv-heads, G=4 (32 query heads), C=64.
  q = (x @ Wq) / sqrt(32);  kv = x @ Wkv;  k, v = split(kv)
  per (b, h, g): S = Qg K^T;  A = softmax(S);  O = A V
  out = concat_heads(O) @ Wp + bp

Sharding: 8 cores = 2 batches x 4 query-chunks of 512 rows. Each core
computes K/V for its whole batch (duplicated within the 4-core group --
no collectives), attention for its 512 query rows over all 32 heads,
and its 512 rows of the output projection. Host concatenates.

Layouts (per core):
  xT   [d, n]  : x transposed, built via PE transpose (fp32)
  Q^T  [j, n]  : lhsT = Wq[d-block, j-block], rhs = xT[d-block, nq]
  K^T  [j, n]  : lhsT = Wkv[d-block, j-block], rhs = xT[d-block, n]
  V~   [n, 65] : per head: V columns + a ones column (gives softmax
                 denominators for free in the PV matmul, psum row 64)
  S^T  [s, q]  : lhsT = K^T[c, s-block], rhs = Q^T[c, q]  (contraction c=64)
  E^T  = exp(S^T / sqrt(32))  on ACT, scale folded into the activation
  O'^T [65, q] : lhsT = V~[s-block, 65], rhs = E^T[s-block, q], accum over s
  o^T  [j, q]  : O'^T rows 0:64 * recip(row 64), broadcast via tiny DMA
  out  [q, d]  : lhsT = o^T[j-block, q-block], rhs = Wp[j-block, d-chunk],
                 bias added during psum evacuation (DMA-broadcast bp)
"""

import numpy as np
from contextlib import ExitStack

import concourse.bass as bass
import concourse.tile as tile
from concourse import bacc, mybir
from concourse.bass_utils import run_bass_kernel_spmd
from concourse.masks import make_identity

P = 128
B, N, D = 2, 2048, 2048
H, G, C = 8, 4, 64
NQ = 512                      # query rows per core
DB = D // P                   # 16 d-blocks
NB = N // P                   # 16 seq blocks
QB = NQ // P                  # 4 query blocks
SCALE = float(1.0 / np.sqrt(H * G))
F32 = mybir.dt.float32
F32R = mybir.dt.float32r
AF = mybir.ActivationFunctionType
BF16 = mybir.dt.bfloat16

USE_F32R = True


def _r(ap):
    if ap.dtype == F32 and USE_F32R:
        return ap.bitcast(F32R)
    return ap


def build_program(n_cores=8, phases="ABCD"):
    nc = bacc.Bacc("TRN2", target_bir_lowering=False, debug=False,
                   num_devices=n_cores)
    xb = nc.dram_tensor("xb", [N, D], F32, kind="ExternalInput").ap()
    xq = nc.dram_tensor("xq", [NQ, D], F32, kind="ExternalInput").ap()
    wq = nc.dram_tensor("wq", [D, D], BF16, kind="ExternalInput").ap()
    wkv = nc.dram_tensor("wkv", [D, 2 * H * C], BF16, kind="ExternalInput").ap()
    wp = nc.dram_tensor("wp", [D, D], BF16, kind="ExternalInput").ap()
    bp = nc.dram_tensor("bp", [D], F32, kind="ExternalInput").ap()
    out = nc.dram_tensor("out", [NQ, D], F32, kind="ExternalOutput").ap()

    with tile.TileContext(nc) as tc, ExitStack() as top:
        # ---- persistent stores ----
        store = top.enter_context(tc.tile_pool(name="store", bufs=1))
        QT = store.tile([P, DB, NQ], F32R, tag="QT")       # [j, n] 32KB/part
        KT = store.tile([P, H * C // P, N], F32R, tag="KT")  # [j, n] 32KB/part
        Vst = store.tile([P, NB, H, C + 1], F32R, tag="Vst")  # [n, h, 65]
        OT = store.tile([P, DB, NQ], BF16, tag="OT")       # o^T [j, q]
        ident = store.tile([P, P], F32, tag="ident")
        make_identity(nc, ident[:])
        ones = store.tile([P, 1], F32, tag="ones")
        nc.gpsimd.memset(ones[:], 1.0)
        nc.vector.tensor_copy(                            # ones column (f32r)
            Vst[:, :, :, C:C + 1],
            ones[:, None, None, :].to_broadcast((P, NB, H, 1)))
        bpb = store.tile([P, D], F32, tag="bpb")
        nc.sync.dma_start(bpb[:], bp[None, :].to_broadcast((P, D)))

        # ---- phase A: transpose xq, project Q^T ----
        with ExitStack() as ctx:
          if 'A' in phases:
              xqT_p = ctx.enter_context(tc.tile_pool(name="xqT", bufs=1))
              xqT = xqT_p.tile([P, DB, NQ], BF16, tag="xqT")
              with ExitStack() as tctx:
                  xrow = tctx.enter_context(tc.tile_pool(name="xrow", bufs=2))
                  tpsum = tctx.enter_context(
                      tc.tile_pool(name="tpsum", bufs=2, space="PSUM"))
                  for qb in range(QB):
                      xt = xrow.tile([P, D], F32, tag="xrow")
                      nc.sync.dma_start(xt[:], xq[qb * P:(qb + 1) * P, :])
                      for db4 in range(DB // 4):
                          tp = tpsum.tile([P, 4, P], F32, tag="tp")
                          for i in range(4):
                              nc.tensor.transpose(
                                  tp[:, i, :],
                                  xt[:, (db4 * 4 + i) * P:(db4 * 4 + i + 1) * P],
                                  ident[:])
                          nc.vector.tensor_copy(
                              xqT[:, db4 * 4:db4 * 4 + 4,
                                  qb * P:(qb + 1) * P], tp[:])
              wq_p = ctx.enter_context(tc.tile_pool(name="wq", bufs=2))
              qpsum = ctx.enter_context(
                  tc.tile_pool(name="qpsum", bufs=8, space="PSUM"))
              # Q^T stored g-major: block bq = g*4 + h//2, row off (h%2)*64 + c.
              # This aligns Q^T's partition offset with K^T's for every (h, g).
              # The host pre-permutes Wq columns to the same g-major order, so
              # stationary slices stay contiguous.
              for half in range(2):          # wq streamed twice, 8 psums/group
                  psums = [qpsum.tile([P, NQ], F32, tag="qpsum",
                                      name=f"qps{half}_{i}") for i in range(8)]
                  for db in range(DB):
                      wt = wq_p.tile([P, D], BF16, tag="wq")
                      eng = nc.sync if db % 2 == 0 else nc.scalar
                      eng.dma_start(wt[:], wq[db * P:(db + 1) * P, :])
                      for i in range(8):
                          bq = half * 8 + i
                          nc.tensor.matmul(
                              psums[i][:], _r(wt[:, bq * P:(bq + 1) * P]),
                              _r(xqT[:, db, :]),
                              start=(db == 0), stop=(db == DB - 1))
                  for i in range(8):
                      nc.vector.tensor_copy(QT[:, half * 8 + i, :], psums[i][:])

        # ---- phase B: transpose xb chunk-wise, project K^T and V~ ----
        with ExitStack() as ctx:
          if 'B' in phases:
              xrow = ctx.enter_context(tc.tile_pool(name="xrowb", bufs=2))
              tpsum = ctx.enter_context(
                  tc.tile_pool(name="tpsumb", bufs=2, space="PSUM"))
              xbT_p = ctx.enter_context(tc.tile_pool(name="xbT", bufs=2))
              wkv_p = ctx.enter_context(tc.tile_pool(name="wkv", bufs=3))
              kvpsum = ctx.enter_context(
                  tc.tile_pool(name="kvpsum", bufs=6, space="PSUM"))

              for ch in range(N // NQ):          # 4 chunks of 512 seq rows
                  xbT = xbT_p.tile([P, DB, NQ], BF16, tag="xbT")
                  for qb in range(QB):
                      xt = xrow.tile([P, D], F32, tag="xrowb")
                      nc.scalar.dma_start(
                          xt[:], xb[ch * NQ + qb * P:ch * NQ + (qb + 1) * P, :])
                      for db4 in range(DB // 4):
                          tp = tpsum.tile([P, 4, P], F32, tag="tpb")
                          for i in range(4):
                              nc.tensor.transpose(
                                  tp[:, i, :],
                                  xt[:, (db4 * 4 + i) * P:(db4 * 4 + i + 1) * P],
                                  ident[:])
                          nc.vector.tensor_copy(
                              xbT[:, db4 * 4:db4 * 4 + 4,
                                  qb * P:(qb + 1) * P], tp[:])
                  # K^T: 4 j-blocks x 512 n, accumulate over d
                  kps = [kvpsum.tile([P, NQ], F32, tag="kvp", name=f"kps{ch}_{i}") for i in range(4)]
                  for db in range(DB):
                      wt = wkv_p.tile([P, H * C], BF16, tag="wkvk")
                      eng = nc.sync if db % 2 == 0 else nc.scalar
                      eng.dma_start(wt[:], wkv[db * P:(db + 1) * P, :H * C])
                      for jb in range(4):
                          nc.tensor.matmul(
                              kps[jb][:], _r(wt[:, jb * P:(jb + 1) * P]),
                              _r(xbT[:, db, :]),
                              start=(db == 0), stop=(db == DB - 1))
                  for jb in range(4):
                      nc.vector.tensor_copy(KT[:, jb, ch * NQ:(ch + 1) * NQ],
                                            kps[jb][:])
                  # V: 4 n-blocks x 512 j, accumulate over d
                  vps = [kvpsum.tile([P, NQ], F32, tag="kvp", name=f"vps{ch}_{i}") for i in range(4)]
                  for db in range(DB):
                      wt = wkv_p.tile([P, H * C], BF16, tag="wkvv")
                      eng = nc.sync if db % 2 == 0 else nc.scalar
                      eng.dma_start(wt[:], wkv[db * P:(db + 1) * P, H * C:])
                      for nb4 in range(4):
                          nc.tensor.matmul(
                              vps[nb4][:], _r(xbT[:, db, nb4 * P:(nb4 + 1) * P]),
                              _r(wt[:]),
                              start=(db == 0), stop=(db == DB - 1))
                  for nb4 in range(4):
                      sb = ch * 4 + nb4
                      for h in range(H):
                          nc.vector.tensor_copy(
                              Vst[:, sb, h, :C],
                              vps[nb4][:, h * C:(h + 1) * C])

        # ---- phase C: attention per (h, g) ----
        with ExitStack() as ctx:
          if 'C' in phases:
              qkpsum = ctx.enter_context(
                  tc.tile_pool(name="qkpsum", bufs=5, space="PSUM"))
              pvpsum = ctx.enter_context(
                  tc.tile_pool(name="pvpsum", bufs=2, space="PSUM"))
              e_p = ctx.enter_context(tc.tile_pool(name="epool", bufs=24))
              rec_p = ctx.enter_context(tc.tile_pool(name="rec", bufs=3))
              rb_p = ctx.enter_context(tc.tile_pool(name="rb", bufs=3))
              dram_p = ctx.enter_context(
                  tc.tile_pool(name="dramrec", bufs=4, space="DRAM"))

              ot_p = ctx.enter_context(tc.tile_pool(name="otmp", bufs=3))
              for h in range(H):
                  for g in range(G):
                      off = (h % 2) * C               # K^T and Q^T row offset
                      kt_jb = h // 2
                      qt_jb = g * 4 + h // 2          # g-major Q^T block
                      e_tiles = []
                      for sb in range(NB):
                          qk = qkpsum.tile([P, NQ], F32, tag="qk")
                          nc.tensor.matmul(
                              qk[:],
                              _r(KT[off:off + C, kt_jb, sb * P:(sb + 1) * P]),
                              _r(QT[off:off + C, qt_jb, :]),
                              start=True, stop=True)
                          et = e_p.tile([P, NQ], F32R, tag="E")
                          nc.scalar.activation(et[:], qk[:], AF.Exp, scale=SCALE)
                          e_tiles.append(et)
                      pv = pvpsum.tile([C + 1, NQ], F32, tag="pv")
                      for sb in range(NB):
                          nc.tensor.matmul(
                              pv[:], _r(Vst[:, sb, h, :]), _r(e_tiles[sb][:]),
                              start=(sb == 0), stop=(sb == NB - 1))
                      rec = rec_p.tile([C + 1, NQ], F32, tag="rec")
                      nc.vector.reciprocal(rec[C:C + 1, :], pv[C:C + 1, :])
                      # partition-broadcast rec via a DRAM bounce (DMA cannot
                      # read SBUF with zero partition step, DRAM is fine)
                      recd = dram_p.tile([1, NQ], F32, tag="recd")
                      nc.sync.dma_start(recd[:], rec[C:C + 1, :])
                      rb = rb_p.tile([C, NQ], F32, tag="rb")
                      nc.sync.dma_start(rb[:], recd[:].to_broadcast((C, NQ)))
                      # o^T rows for (h,g) live at j = h*G*C + g*C (+64 for odd
                      # g); DVE can't shift partitions, so odd halves go via a
                      # small SBUF->SBUF DMA.
                      oj = h * G * C + g * C
                      o_jb, o_off = oj // P, oj % P
                      if o_off == 0:
                          nc.vector.tensor_mul(OT[:C, o_jb, :], pv[:C, :], rb[:])
                      else:
                          ot = ot_p.tile([C, NQ], BF16, tag="otmp")
                          nc.vector.tensor_mul(ot[:], pv[:C, :], rb[:])
                          nc.sync.dma_start(OT[o_off:o_off + C, o_jb, :], ot[:])

        # ---- phase D: output projection + bias ----
        with ExitStack() as ctx:
          if 'D' in phases:
              wp_p = ctx.enter_context(tc.tile_pool(name="wp", bufs=2))
              opsum = ctx.enter_context(
                  tc.tile_pool(name="opsum", bufs=3, space="PSUM"))
              osb_p = ctx.enter_context(tc.tile_pool(name="osb", bufs=3))

              for ob in range(4):                 # output col chunks of 512
                  wpt = wp_p.tile([P, DB, NQ], BF16, tag="wp")
                  for jb in range(DB):
                      eng = nc.sync if jb % 2 == 0 else nc.scalar
                      eng.dma_start(
                          wpt[:, jb, :],
                          wp[jb * P:(jb + 1) * P, ob * NQ:(ob + 1) * NQ])
                  for qb in range(QB):
                      ps = opsum.tile([P, NQ], F32, tag="op")
                      for jb in range(DB):
                          nc.tensor.matmul(
                              ps[:], _r(OT[:, jb, qb * P:(qb + 1) * P]),
                              _r(wpt[:, jb, :]),
                              start=(jb == 0), stop=(jb == DB - 1))
                      osb = osb_p.tile([P, NQ], F32, tag="osb")
                      nc.vector.tensor_add(osb[:], ps[:],
                                           bpb[:, ob * NQ:(ob + 1) * NQ])
                      nc.sync.dma_start(
                          out[qb * P:(qb + 1) * P, ob * NQ:(ob + 1) * NQ],
                          osb[:])

    nc.compile()
    return nc


_nc_cache = None


def kernel(x, Wq, Wkv, Wp, bp):
    global _nc_cache
    if _nc_cache is None:
        _nc_cache = build_program()
    nc = _nc_cache
    x = np.ascontiguousarray(np.asarray(x, dtype=np.float32))
    import ml_dtypes
    # permute Wq columns to g-major head order (see build_program phase A)
    Wq = np.ascontiguousarray(
        np.asarray(Wq, dtype=np.float32)
        .reshape(D, H, G, C).transpose(0, 2, 1, 3).reshape(D, D)
        .astype(ml_dtypes.bfloat16))
    Wkv = np.asarray(Wkv, dtype=np.float32).astype(ml_dtypes.bfloat16)
    Wp = np.asarray(Wp, dtype=np.float32).astype(ml_dtypes.bfloat16)
    bp = np.ascontiguousarray(np.asarray(bp, dtype=np.float32))

    in_maps = []
    for c in range(8):
        b, qc = c // 4, c % 4
        in_maps.append({
            "xb": x[b],
            "xq": x[b, qc * NQ:(qc + 1) * NQ],
            "wq": Wq, "wkv": Wkv, "wp": Wp, "bp": bp,
        })
    res = run_bass_kernel_spmd(nc, in_maps, list(range(8)))
    outp = np.empty((B, N, D), np.float32)
    for c in range(8):
        outp[c // 4, (c % 4) * NQ:(c % 4 + 1) * NQ] = res.results[c]["out"]
    return outp



# revision 5
# speedup vs baseline: 1.6466x; 1.6466x over previous
"""Trainium2 Bass kernel for GroupedQuerySelfAttention (head-TP sharding).

Problem: B=2, N=2048, D=2048, H=8 kv-heads, G=4 (32 query heads), C=64.
  q = (x @ Wq) / sqrt(32);  kv = x @ Wkv;  k, v = split(kv)
  per (b, h, g): S = Qg K^T;  A = softmax(S);  O = A V
  out = concat_heads(O) @ Wp + bp

Sharding: 8 cores = 2 batches x 4 kv-head-pairs (tensor parallel over the
kv-head dim, per the sharding hint). Each core owns 2 kv heads and their
8 (h, g) query heads over the full sequence: it projects only its slice
of Q/K/V (no duplicated projection work), runs attention for its 8 pairs,
and computes a partial output projection over its 512 o-columns. The
all-reduce after the output projection is folded into the host gather
(partials are summed on host; bias added there too).

Per-core layouts (everything bf16 except psum):
  XT  [d, n]   x^T chunks, built via PE transpose of ACT-cast bf16 rows
  QT  [128,4,2048]  Q^T: pair (g, l) lives at rows l*64+c, block g
  KT  [128,2048]    K^T: local head l at rows l*64+c
  VT  [128,16,2,65] V rows [n, c] per (sb, l) + ones column (softmax denom)
  S^T psum [128 s, 1024 q] <- 2 matmuls; one wide exp per psum (ACT)
  PV  psum [128 q, 65] accumulated over sb: O rows + denominator col
  OT  [128,4,2048]  o^T via PE transpose of normalized O blocks
  out [2048, 2048] bf16 partial = o^T.T @ Wp (host sums 4 partials + bias)
"""

import numpy as np
from contextlib import ExitStack

import concourse.bass as bass
import concourse.tile as tile
from concourse import bacc, mybir
from concourse.bass_utils import run_bass_kernel_spmd
from concourse.masks import make_identity

P = 128
B, N, D = 2, 2048, 2048
H, G, C = 8, 4, 64
DB = D // P                    # 16 d-blocks
NB = N // P                    # 16 seq blocks
CH = 4                         # n chunks of 512
SCALE = float(1.0 / np.sqrt(H * G))
F32 = mybir.dt.float32
BF16 = mybir.dt.bfloat16
AF = mybir.ActivationFunctionType


def build_program(n_cores=8):
    nc = bacc.Bacc("TRN2", target_bir_lowering=False, debug=False,
                   num_devices=n_cores)
    xb = nc.dram_tensor("xb", [N, D], F32, kind="ExternalInput").ap()
    wq = nc.dram_tensor("wq", [D, 512], BF16, kind="ExternalInput").ap()
    wkv = nc.dram_tensor("wkv", [D, 256], BF16, kind="ExternalInput").ap()
    wp = nc.dram_tensor("wp", [512, D], BF16, kind="ExternalInput").ap()
    out = nc.dram_tensor("out", [N, D], BF16, kind="ExternalOutput").ap()

    with tile.TileContext(nc) as tc, ExitStack() as top:
        # ---- persistent stores ----
        store = top.enter_context(tc.tile_pool(name="store", bufs=1))
        QT = store.tile([P, 4, N], BF16, tag="QT")
        KT = store.tile([P, N], BF16, tag="KT")
        VT = store.tile([P, NB, 2, C + 1], BF16, tag="VT")
        # WQ dies (last Q projection) before OT is born (first o^T
        # transpose), so they share one slot via the same tag
        wqot = top.enter_context(tc.tile_pool(name="wqot", bufs=1))
        WQ = wqot.tile([P, DB, 512], BF16, tag="wqot")
        WKV = store.tile([P, DB, 256], BF16, tag="WKV")
        WP = store.tile([P, 4, N], BF16, tag="WP")
        identB = store.tile([P, P], BF16, tag="identB")
        make_identity(nc, identB[:])
        nc.gpsimd.memset(VT[:, :, :, C:C + 1], 1.0)

        # weight loads on the gpsimd DGE queue (cheap issue, early)
        for db in range(DB):
            nc.gpsimd.dma_start(WKV[:, db, :], wkv[db * P:(db + 1) * P, :])
        for db in range(DB):
            nc.gpsimd.dma_start(WQ[:, db, :], wq[db * P:(db + 1) * P, :])
        for jb in range(4):
            nc.gpsimd.dma_start(WP[:, jb, :], wp[jb * P:(jb + 1) * P, :])

        # persistent psum pool: one bank-sized f32 tag shared by the
        # K/Q-projection accumulators and the output-projection accumulators
        bank = top.enter_context(tc.tile_pool(name="bank", bufs=2,
                                              space="PSUM"))

        XT_pool = top.enter_context(tc.tile_pool(name="XT", bufs=2))
        xt_tiles = {}

        def phase_T(ch):
            """transpose x rows of chunk ch into XT[ch] (bf16)."""
            xt = XT_pool.tile([P, DB, 512], BF16, tag="XT", name=f"xt{ch}")
            xt_tiles[ch] = xt
            for rt in range(4):
                r0 = ch * 512 + rt * P
                xrow = ab_x.tile([P, D], F32, tag="xrow", name=f"xr{ch}_{rt}")
                eng = nc.sync if rt % 2 == 0 else nc.scalar
                eng.dma_start(xrow[:], xb[r0:r0 + P, :])
                xbf = ab_x.tile([P, D], BF16, tag="xbf", name=f"xc{ch}_{rt}")
                nc.scalar.activation(xbf[:], xrow[:], AF.Copy)
                for dq in range(4):
                    tp = bank.tile([P, 4, P], BF16, tag="bank",
                                   name=f"tp{ch}_{rt}_{dq}")
                    for i in range(4):
                        nc.tensor.transpose(
                            tp[:, i, :],
                            xbf[:, (dq * 4 + i) * P:(dq * 4 + i + 1) * P],
                            identB[:])
                    nc.vector.tensor_copy(
                        xt[:, dq * 4:dq * 4 + 4, rt * P:(rt + 1) * P], tp[:])

        def phase_K(ch):
            xt = xt_tiles[ch]
            kps = bank.tile([P, 512], F32, tag="bank", name=f"kps{ch}")
            for db in range(DB):
                nc.tensor.matmul(kps[:], WKV[:, db, 0:P], xt[:, db, :],
                                 start=(db == 0), stop=(db == DB - 1))
            nc.vector.tensor_copy(KT[:, ch * 512:(ch + 1) * 512], kps[:])

        def phase_V(ch):
            xt = xt_tiles[ch]
            vps = bank.tile([P, 4, 2, C], F32, tag="bank", name=f"vps{ch}")
            for nb4 in range(4):
                for db in range(DB):
                    nc.tensor.matmul(vps[:, nb4, :, :],
                                     xt[:, db, nb4 * P:(nb4 + 1) * P],
                                     WKV[:, db, P:2 * P],
                                     start=(db == 0), stop=(db == DB - 1))
            for nb4 in range(4):
                sb = ch * 4 + nb4
                nc.vector.tensor_copy(VT[:, sb, :, 0:C], vps[:, nb4, :, :])

        def phase_Q(ch):
            xt = xt_tiles.pop(ch)
            for g in range(4):
                qps = bank.tile([P, 512], F32, tag="bank", name=f"qps{ch}_{g}")
                for db in range(DB):
                    nc.tensor.matmul(qps[:], WQ[:, db, g * P:(g + 1) * P],
                                     xt[:, db, :],
                                     start=(db == 0), stop=(db == DB - 1))
                nc.vector.tensor_copy(QT[:, g, ch * 512:(ch + 1) * 512],
                                      qps[:])

        # ---- phase AB prologue: transposes + KV for chunks 0,1; Q 0,1 ----
        ab_scope = ExitStack()
        ab_x = ab_scope.enter_context(
            tc.tile_pool(name="abx", bufs=2, side="right"))
        for ch in range(2):
            phase_T(ch)
            phase_K(ch)
            phase_V(ch)
        phase_Q(0)
        phase_Q(1)

        # ---- attention + output projection, software-pipelined ----
        cps = top.enter_context(
            tc.tile_pool(name="cps", bufs=1, space="PSUM"))
        csb = top.enter_context(tc.tile_pool(name="csb", bufs=2))
        ost_tiles = {}
        e_tiles = {}
        ot_holder = {}

        def get_OT():
            if "OT" not in ot_holder:
                ot_holder["OT"] = wqot.tile([P, 4, N], BF16, tag="wqot",
                                            name="OT")
            return ot_holder["OT"]

        def emit_qk(qh, p, sb_list):
            g, l = p // 2, p % 2
            key = (qh, p)
            if key not in e_tiles:
                e_tiles[key] = csb.tile([P, NB, 1024], BF16, tag="E",
                                        name=f"E{qh}_{p}")
            E = e_tiles[key]
            o = l * C
            for sb in sb_list:
                qk = cps.tile([P, 1024], F32, tag="qk", bufs=2,
                              name=f"qk{qh}_{p}_{sb}")
                for h2 in range(2):
                    nc.tensor.matmul(
                        qk[:, h2 * 512:(h2 + 1) * 512],
                        KT[o:o + C, sb * P:(sb + 1) * P],
                        QT[o:o + C, g, qh * 1024 + h2 * 512:
                           qh * 1024 + (h2 + 1) * 512],
                        start=True, stop=True)
                nc.scalar.activation(E[:, sb, :], qk[:], AF.Exp, scale=SCALE)

        def emit_pv(qh, p):
            g, l = p // 2, p % 2
            E = e_tiles.pop((qh, p))
            if qh not in ost_tiles:
                ost_tiles[qh] = csb.tile([P, 8, 8, C], BF16, tag="Ost",
                                         name=f"Ost{qh}")
            Ost = ost_tiles[qh]
            for qb in range(8):
                pv = cps.tile([P, C + 1], F32, tag="pv", bufs=1,
                              name=f"pv{qh}_{p}_{qb}")
                for sb in range(NB):
                    nc.tensor.matmul(pv[:], E[:, sb, qb * P:(qb + 1) * P],
                                     VT[:, sb, l, :],
                                     start=(sb == 0), stop=(sb == NB - 1))
                rec = csb.tile([P, 1], F32, tag="rec", bufs=2,
                               name=f"rec{qh}_{p}_{qb}")
                nc.vector.reciprocal(rec[:], pv[:, C:C + 1])
                nc.vector.tensor_scalar_mul(Ost[:, qb, p, :], pv[:, 0:C],
                                            rec[:])

        def emit_ot_trans(qh):
            Ost = ost_tiles.pop(qh)
            OT = get_OT()
            for qb in range(8):
                tp2 = cps.tile([P, 4, P], BF16, tag="tp2", bufs=1,
                               name=f"tp2_{qh}_{qb}")
                for g in range(4):
                    nc.tensor.transpose(tp2[:, g, :],
                                        Ost[:, qb, 2 * g:2 * g + 2, :],
                                        identB[:])
                for g in range(4):
                    nc.vector.tensor_copy(
                        OT[:, g, qh * 1024 + qb * P:qh * 1024 + (qb + 1) * P],
                        tp2[:, g, :])

        def emit_outproj(qh, qb):
            OT = get_OT()
            qw = qh * 1024 + qb * P
            for dc in range(4):
                ops = bank.tile([P, 512], F32, tag="bank",
                                name=f"ops{qh}_{qb}_{dc}")
                for jb in range(4):
                    nc.tensor.matmul(ops[:], OT[:, jb, qw:qw + P],
                                     WP[:, jb, dc * 512:(dc + 1) * 512],
                                     start=(jb == 0), stop=(jb == 3))
                osb = csb.tile([P, 512], BF16, tag="osb", bufs=2,
                               name=f"osb{qh}_{qb}_{dc}")
                nc.vector.tensor_copy(osb[:], ops[:])
                nc.sync.dma_start(out[qw:qw + P, dc * 512:(dc + 1) * 512],
                                  osb[:])

        # pair 0 of qh 0: interleave remaining AB chunks into its sb loop
        emit_qk(0, 0, range(0, 8))
        phase_T(2)
        phase_K(2)
        emit_qk(0, 0, range(8, 12))
        phase_V(2)
        phase_T(3)
        phase_K(3)
        emit_qk(0, 0, range(12, 16))
        phase_V(3)
        ab_scope.close()

        for qh in range(2):
            for p in range(8):
                if not (qh == 0 and p == 0):
                    emit_qk(qh, p, range(NB))
                # trailing work interleaved behind the ACT-bound QK stream
                if qh == 0:
                    if p == 2:
                        phase_Q(2)
                    elif p == 5:
                        phase_Q(3)
                else:
                    if p == 1:
                        emit_ot_trans(0)
                    elif p >= 2:
                        emit_outproj(0, 2 * (p - 2))
                        emit_outproj(0, 2 * (p - 2) + 1)
                # pipelined PV of the previous pair
                if p > 0:
                    emit_pv(qh, p - 1)
                elif qh == 1:
                    emit_pv(0, 7)
        emit_pv(1, 7)
        emit_outproj(0, 6)
        emit_outproj(0, 7)
        emit_ot_trans(1)
        for qb in range(8):
            emit_outproj(1, qb)

    nc.compile()
    return nc


_nc_cache = None

# query-head column order per core: j_local = g*128 + l*64 + c maps to
# original column (2*hp + l)*G*C + g*C + c  (same permutation for Wq cols
# and Wp rows)
def _perm(hp):
    idx = np.empty(512, np.int64)
    for g in range(G):
        for l in range(2):
            base = (2 * hp + l) * G * C + g * C
            idx[g * 128 + l * 64:g * 128 + l * 64 + 64] = np.arange(
                base, base + C)
    return idx


def kernel(x, Wq, Wkv, Wp, bp):
    global _nc_cache
    if _nc_cache is None:
        _nc_cache = build_program()
    nc = _nc_cache
    import ml_dtypes
    x = np.ascontiguousarray(np.asarray(x, dtype=np.float32))
    Wq = np.asarray(Wq, dtype=np.float32)
    Wkv = np.asarray(Wkv, dtype=np.float32)
    Wp = np.asarray(Wp, dtype=np.float32)
    bp = np.asarray(bp, dtype=np.float32)

    in_maps = []
    for c in range(8):
        b, hp = c // 4, c % 4
        idx = _perm(hp)
        wq_c = np.ascontiguousarray(Wq[:, idx]).astype(ml_dtypes.bfloat16)
        wkv_c = np.ascontiguousarray(np.concatenate(
            [Wkv[:, hp * P:(hp + 1) * P],
             Wkv[:, H * C + hp * P:H * C + (hp + 1) * P]],
            axis=1)).astype(ml_dtypes.bfloat16)
        wp_c = np.ascontiguousarray(Wp[idx, :]).astype(ml_dtypes.bfloat16)
        in_maps.append({"xb": x[b], "wq": wq_c, "wkv": wkv_c, "wp": wp_c})
    res = run_bass_kernel_spmd(nc, in_maps, list(range(8)))
    outp = np.empty((B, N, D), np.float32)
    for b in range(B):
        acc = np.zeros((N, D), np.float32)
        for hp in range(4):
            acc += np.asarray(res.results[b * 4 + hp]["out"],
                              dtype=np.float32)
        outp[b] = acc + bp
    return outp


# revision 28
# speedup vs baseline: 1.8329x; 1.1132x over previous
"""Trainium2 Bass kernel for GroupedQuerySelfAttention (head-TP sharding).

Problem: B=2, N=2048, D=2048, H=8 kv-heads, G=4 (32 query heads), C=64.
  q = (x @ Wq) / sqrt(32);  kv = x @ Wkv;  k, v = split(kv)
  per (b, h, g): S = Qg K^T;  A = softmax(S);  O = A V
  out = concat_heads(O) @ Wp + bp

Sharding: 8 cores = 2 batches x 4 kv-head-pairs (tensor parallel over the
kv-head dim, per the sharding hint). Each core owns 2 kv heads and their
8 (h, g) query heads over the full sequence: it projects only its slice
of Q/K/V (no duplicated projection work), runs attention for its 8 pairs,
and computes a partial output projection over its 512 o-columns. The
all-reduce after the output projection is folded into the host gather
(partials are summed on host; bias added there too).

Per-core layouts (everything bf16 except psum):
  XT  [128,4,16,512]  x^T via transposing DMA (host pre-casts x to bf16)
  QT  [128,4,2048]  Q^T: pair (g, l) lives at rows l*64+c, block g
  KT  [128,2048]    K^T: local head l at rows l*64+c
  VT  [128,16,2,65] V rows [n, c] per (sb, l) + ones column (softmax denom)
  S^T psum [128 s, <=1024 q] <- 512-row matmuls; one wide exp per psum
  PV  psum [128 q, 4, 65] accumulated over sb: O rows + denominator col
  OT  [128,4,2048]  o^T via PE transpose of normalized O blocks
  out [2048, 2048] bf16 partial = o^T.T @ Wp (host sums 4 partials + bias)

The emission order hand-interleaves projection work into the gaps of the
ACT-bound exp stream (ACT is the second-busiest engine at ~267us; PE
~325us) and starts attention on the first 512 query columns as soon as
K-chunk 0 and Q-chunk 0 are projected.
"""

import numpy as np
from contextlib import ExitStack

import concourse.bass as bass
import concourse.tile as tile
from concourse import bacc, mybir
from concourse.bass_utils import run_bass_kernel_spmd
from concourse.masks import make_identity

P = 128
B, N, D = 2, 2048, 2048
H, G, C = 8, 4, 64
DB = D // P                    # 16 d-blocks
NB = N // P                    # 16 seq blocks
SCALE = float(1.0 / np.sqrt(H * G))
F32 = mybir.dt.float32
BF16 = mybir.dt.bfloat16
AF = mybir.ActivationFunctionType


def build_program(n_cores=8):
    nc = bacc.Bacc("TRN2", target_bir_lowering=False, debug=False,
                   num_devices=n_cores)
    xb = nc.dram_tensor("xb", [N, D], BF16, kind="ExternalInput").ap()
    wq = nc.dram_tensor("wq", [DB, P, 512], BF16, kind="ExternalInput").ap()
    wkv = nc.dram_tensor("wkv", [DB, P, 256], BF16,
                         kind="ExternalInput").ap()
    wp = nc.dram_tensor("wp", [4, P, D], BF16, kind="ExternalInput").ap()
    out = nc.dram_tensor("out", [N, D], BF16, kind="ExternalOutput").ap()

    with tile.TileContext(nc) as tc, ExitStack() as top:
        # ---- persistent stores ----
        store = top.enter_context(tc.tile_pool(name="store", bufs=1))
        QT = store.tile([P, 4, N], BF16, tag="QT")
        KT = store.tile([P, N], BF16, tag="KT")
        VT = store.tile([P, NB, 2, C + 1], BF16, tag="VT")
        # WQ dies (last Q projection) before OT is born (first o^T
        # transpose), so they share one slot via the same tag
        wqot = top.enter_context(tc.tile_pool(name="wqot", bufs=1))
        WQ = wqot.tile([P, DB, 512], BF16, tag="wqot")
        WKV = store.tile([P, DB, 256], BF16, tag="WKV")
        WP = store.tile([P, 4, N], BF16, tag="WP")
        identB = store.tile([P, P], BF16, tag="identB")
        make_identity(nc, identB[:])
        nc.gpsimd.memset(VT[:, :, :, C:C + 1], 1.0)

        # weight loads lead the two hw DGE queues (wkv is needed first);
        # x^T is built by the transposing DMA, split across both queues
        # chunk-major so chunk 0 lands first; wp rides the gpsimd path
        XT_pool = top.enter_context(tc.tile_pool(name="XT", bufs=3))
        xbf_pool = top.enter_context(
            tc.tile_pool(name="xbf", bufs=4, side="right"))
        xt_tiles = {}

        def xt_slot(ch):
            if ch not in xt_tiles:
                xt_tiles[ch] = XT_pool.tile([P, DB, 512], BF16, tag="XT",
                                            name=f"xt{ch}")
            return xt_tiles[ch]

        def load_xt(ch):
            """chunks 2/3: transposing DMA (emitted late, queues idle)"""
            xt = xt_slot(ch)
            for db in range(DB):
                eng = nc.sync if db % 2 == 0 else nc.scalar
                eng.dma_start_transpose(
                    xt[:, db, :],
                    xb[ch * 512:(ch + 1) * 512, db * P:(db + 1) * P])

        xbf_tiles = {}

        def load_rows(ch):
            """chunks 0/1: plain row loads (cheap on the serial DMA path)"""
            for rt in range(4):
                r0 = ch * 512 + rt * P
                xr = xbf_pool.tile([P, D], BF16, tag="xbf",
                                   name=f"xr{ch}_{rt}")
                eng = nc.sync if rt % 2 == 0 else nc.scalar
                eng.dma_start(xr[:], xb[r0:r0 + P, :])
                xbf_tiles[(ch, rt)] = xr

        # persistent psum pool: one bank-sized f32 tag shared by the
        # K/Q/V-projection accumulators and the output-projection
        # accumulators (2 banks)
        bank = top.enter_context(tc.tile_pool(name="bank", bufs=2,
                                              space="PSUM"))

        def phase_K(ch):
            xt = xt_slot(ch)
            kps = bank.tile([P, 512], F32, tag="bank", name=f"kps{ch}")
            for db in range(DB):
                nc.tensor.matmul(kps[:], WKV[:, db, 0:P], xt[:, db, :],
                                 start=(db == 0), stop=(db == DB - 1))
            nc.vector.tensor_copy(KT[:, ch * 512:(ch + 1) * 512], kps[:])

        def phase_V(ch):
            xt = xt_slot(ch)
            vps = bank.tile([P, 4, 2, C], F32, tag="bank", name=f"vps{ch}")
            for nb4 in range(4):
                for db in range(DB):
                    nc.tensor.matmul(vps[:, nb4, :, :],
                                     xt[:, db, nb4 * P:(nb4 + 1) * P],
                                     WKV[:, db, P:2 * P],
                                     start=(db == 0), stop=(db == DB - 1))
            for nb4 in range(4):
                sb = ch * 4 + nb4
                nc.vector.tensor_copy(VT[:, sb, :, 0:C], vps[:, nb4, :, :])

        def phase_Q(ch, gs=range(4)):
            xt = xt_slot(ch)
            for g in gs:
                qps = bank.tile([P, 512], F32, tag="bank",
                                name=f"qps{ch}_{g}")
                for db in range(DB):
                    nc.tensor.matmul(qps[:], WQ[:, db, g * P:(g + 1) * P],
                                     xt[:, db, :],
                                     start=(db == 0), stop=(db == DB - 1))
                nc.vector.tensor_copy(QT[:, g, ch * 512:(ch + 1) * 512],
                                      qps[:])

        # ---- attention + output projection, software-pipelined ----
        cps = top.enter_context(
            tc.tile_pool(name="cps", bufs=1, space="PSUM"))
        csb = top.enter_context(tc.tile_pool(name="csb", bufs=2))

        def trans_rows(ch, rts=range(4)):
            """PE-transpose row tiles of a chunk into its XT slot"""
            xt = xt_slot(ch)
            for rt in rts:
                xr = xbf_tiles.pop((ch, rt))
                for dq in range(4):
                    tp = cps.tile([P, 4, P], BF16, tag="qk", bufs=2,
                                  name=f"tp{ch}_{rt}_{dq}")
                    for i in range(4):
                        nc.tensor.transpose(
                            tp[:, i, :],
                            xr[:, (dq * 4 + i) * P:(dq * 4 + i + 1) * P],
                            identB[:])
                    nc.vector.tensor_copy(
                        xt[:, dq * 4:dq * 4 + 4, rt * P:(rt + 1) * P],
                        tp[:])
        ost_tiles = {}
        e_tiles = {}
        ot_holder = {}

        def get_OT():
            if "OT" not in ot_holder:
                ot_holder["OT"] = wqot.tile([P, 4, N], BF16, tag="wqot",
                                            name="OT")
            return ot_holder["OT"]

        def emit_qk(qh, p, sb_list, cols=(0, 1024)):
            g, l = p // 2, p % 2
            key = (qh, p)
            if key not in e_tiles:
                e_tiles[key] = csb.tile([P, NB, 1024], BF16, tag="E",
                                        name=f"E{qh}_{p}")
            E = e_tiles[key]
            o = l * C
            c0, c1 = cols
            for sb in sb_list:
                qk = cps.tile([P, c1 - c0], F32, tag="qk", bufs=2,
                              name=f"qk{qh}_{p}_{sb}_{c0}")
                for h2 in range((c1 - c0) // 512):
                    nc.tensor.matmul(
                        qk[:, h2 * 512:(h2 + 1) * 512],
                        KT[o:o + C, sb * P:(sb + 1) * P],
                        QT[o:o + C, g, qh * 1024 + c0 + h2 * 512:
                           qh * 1024 + c0 + (h2 + 1) * 512],
                        start=True, stop=True)
                nc.scalar.activation(E[:, sb, c0:c1], qk[:], AF.Exp,
                                     scale=SCALE)

        def emit_pv(qh, p, halves=range(2)):
            g, l = p // 2, p % 2
            E = e_tiles[(qh, p)]
            if qh not in ost_tiles:
                ost_tiles[qh] = csb.tile([P, 8, 8, C], BF16, tag="Ost",
                                         bufs=1, name=f"Ost{qh}")
            Ost = ost_tiles[qh]
            for h4 in halves:
                pv = cps.tile([P, 4, C + 1], F32, tag="pv", bufs=1,
                              name=f"pv{qh}_{p}_{h4}")
                for qi in range(4):
                    qb = h4 * 4 + qi
                    for sb in range(NB):
                        nc.tensor.matmul(pv[:, qi, :],
                                         E[:, sb, qb * P:(qb + 1) * P],
                                         VT[:, sb, l, :],
                                         start=(sb == 0),
                                         stop=(sb == NB - 1))
                rec = csb.tile([P, 4], F32, tag="rec", bufs=2,
                               name=f"rec{qh}_{p}_{h4}")
                nc.vector.reciprocal(rec[:], pv[:, :, C])
                nc.vector.tensor_mul(
                    Ost[:, h4 * 4:h4 * 4 + 4, p, :], pv[:, :, 0:C],
                    rec[:, :, None].to_broadcast((P, 4, C)))

        def emit_ot_trans(qh, qbs=range(8)):
            Ost = ost_tiles[qh]
            OT = get_OT()
            for qb in qbs:
                tp2 = cps.tile([P, 4, P], BF16, tag="tp2", bufs=1,
                               name=f"tp2_{qh}_{qb}")
                for g in range(4):
                    nc.tensor.transpose(tp2[:, g, :],
                                        Ost[:, qb, 2 * g:2 * g + 2, :],
                                        identB[:])
                for g in range(4):
                    nc.vector.tensor_copy(
                        OT[:, g, qh * 1024 + qb * P:qh * 1024 + (qb + 1) * P],
                        tp2[:, g, :])

        def emit_outproj(qh, qb):
            OT = get_OT()
            qw = qh * 1024 + qb * P
            for half in range(2):
                osb = csb.tile([P, 1024], BF16, tag="osb", bufs=3,
                               name=f"osb{qh}_{qb}_{half}")
                for dc2 in range(2):
                    dc = half * 2 + dc2
                    ops = bank.tile([P, 512], F32, tag="bank",
                                    name=f"ops{qh}_{qb}_{dc}")
                    for jb in range(4):
                        nc.tensor.matmul(ops[:], OT[:, jb, qw:qw + P],
                                         WP[:, jb, dc * 512:(dc + 1) * 512],
                                         start=(jb == 0), stop=(jb == 3))
                    nc.vector.tensor_copy(osb[:, dc2 * 512:(dc2 + 1) * 512],
                                          ops[:])
                eng = nc.sync if half == 0 else nc.scalar
                eng.dma_start(
                    out[qw:qw + P, half * 1024:(half + 1) * 1024], osb[:])

        # ---- main interleaved emission ----
        # preamble DMAs: wkv first (K0 needs it), chunk 0+1 rows, then wq
        # in two halves behind them on the scalar queue (the DMA transfer
        # path is serial in the model, so order matters)
        with tc.high_priority():
            nc.sync.dma_start(WKV[:], wkv[:, :, :].transpose([1, 0, 2]))
            load_rows(0)
            load_rows(1)
            for wh in range(2):
                nc.sync.dma_start(WQ[:, :, wh * 256:(wh + 1) * 256],
                                  wq[:, :, wh * 256:(wh + 1) * 256]
                                  .transpose([1, 0, 2]))
            nc.gpsimd.dma_start(WP[:], wp[:, :, :].transpose([1, 0, 2]))

        warm = cps.tile([P, P], BF16, tag="tp2", bufs=1, name="warm")
        for i in range(175):
            nc.tensor.transpose(warm[:], identB[:], identB[:])
        trans_rows(0)
        phase_K(0)
        trans_rows(1)
        phase_K(1)
        # attention starts on query cols 0:512 (pairs 0/1 need only the
        # g0 slice of Q-chunk 0); the rest of the projections thread into
        # the exp-stream gaps
        phase_Q(0, [0])
        emit_qk(0, 0, range(0, 4), (0, 512))
        phase_V(0)
        emit_qk(0, 0, range(4, 8), (0, 512))
        phase_Q(0, [1])
        emit_qk(0, 1, range(0, 8), (0, 512))
        phase_Q(0, [2])
        phase_Q(1, [0])
        emit_qk(0, 0, range(0, 4), (512, 1024))
        phase_Q(0, [3])
        emit_qk(0, 0, range(4, 8), (512, 1024))
        phase_Q(1, [1])
        emit_qk(0, 1, range(0, 4), (512, 1024))
        load_rows(2)
        trans_rows(2, [0, 1])
        emit_qk(0, 1, range(4, 8), (512, 1024))
        trans_rows(2, [2, 3])
        phase_K(2)
        emit_qk(0, 0, range(8, 12))
        load_rows(3)
        trans_rows(3, [0, 1])
        emit_qk(0, 1, range(8, 12))
        trans_rows(3, [2, 3])
        phase_K(3)
        emit_qk(0, 0, range(12, 16))
        phase_V(1)
        emit_qk(0, 1, range(12, 16))
        phase_V(2)
        phase_V(3)
        emit_pv(0, 0)

        # filler units consumed between QK batches of qh0 pairs 2..7;
        # ordered so each Q slice lands just before the pair that needs it
        fillers = [
            lambda: phase_Q(1, [2]), lambda: phase_Q(1, [3]),
            lambda: phase_Q(2, [0]), lambda: phase_Q(3, [0]),
            lambda: phase_Q(2, [1]), lambda: phase_Q(3, [1]),
            lambda: phase_Q(2, [2]), lambda: phase_Q(3, [2]),
            lambda: phase_Q(2, [3]), lambda: phase_Q(3, [3]),
        ]  # Q-chunk slices land just before the pairs that need them
        for p in range(2, 8):
            emit_qk(0, p, range(0, 8))
            if fillers:
                fillers.pop(0)()
            emit_qk(0, p, range(8, 16))
            if fillers:
                fillers.pop(0)()
            emit_pv(0, p - 1)

        # qh1 pairs with qh0's transposes + output projection as filler
        for p in range(8):
            if p == 7:
                # column-split the last pair so its first PV group can
                # start before the second half of its exps finishes
                emit_qk(1, 7, range(0, 16), (0, 512))
                emit_outproj(0, 6)
                emit_pv(1, 6)
                emit_qk(1, 7, range(0, 16), (512, 1024))
                emit_outproj(0, 7)
                continue
            emit_qk(1, p, range(0, 8))
            if p == 0:
                emit_pv(0, 7)
                emit_qk(1, 0, range(8, 16))
                emit_ot_trans(0)
                continue
            emit_qk(1, p, range(8, 16))
            emit_outproj(0, p - 1)
            if p < 7:
                emit_pv(1, p - 1)

        # tail: last pair's PV, o^T transposes and output projection are
        # pipelined per 4-qb group to keep PE streaming to the end
        emit_pv(1, 7, [0])
        emit_ot_trans(1, range(0, 4))
        emit_outproj(1, 0)
        emit_pv(1, 7, [1])
        emit_outproj(1, 1)
        emit_ot_trans(1, range(4, 6))
        emit_outproj(1, 2)
        emit_ot_trans(1, range(6, 8))
        emit_outproj(1, 3)
        for qb in range(4, 8):
            emit_outproj(1, qb)

    nc.compile()
    return nc


_nc_cache = None

# query-head column order per core: j_local = g*128 + l*64 + c maps to
# original column (2*hp + l)*G*C + g*C + c  (same permutation for Wq cols
# and Wp rows)
def _perm(hp):
    idx = np.empty(512, np.int64)
    for g in range(G):
        for l in range(2):
            base = (2 * hp + l) * G * C + g * C
            idx[g * 128 + l * 64:g * 128 + l * 64 + 64] = np.arange(
                base, base + C)
    return idx


def kernel(x, Wq, Wkv, Wp, bp):
    global _nc_cache
    if _nc_cache is None:
        _nc_cache = build_program()
    nc = _nc_cache
    import ml_dtypes
    x = np.asarray(x, dtype=np.float32)
    xb16 = [np.ascontiguousarray(x[b]).astype(ml_dtypes.bfloat16)
            for b in range(B)]
    Wq = np.asarray(Wq, dtype=np.float32)
    Wkv = np.asarray(Wkv, dtype=np.float32)
    Wp = np.asarray(Wp, dtype=np.float32)
    bp = np.asarray(bp, dtype=np.float32)

    in_maps = []
    for c in range(8):
        b, hp = c // 4, c % 4
        idx = _perm(hp)
        wq_c = np.ascontiguousarray(Wq[:, idx]).astype(
            ml_dtypes.bfloat16).reshape(DB, P, 512)
        wkv_c = np.ascontiguousarray(np.concatenate(
            [Wkv[:, hp * P:(hp + 1) * P],
             Wkv[:, H * C + hp * P:H * C + (hp + 1) * P]],
            axis=1)).astype(ml_dtypes.bfloat16).reshape(DB, P, 256)
        wp_c = np.ascontiguousarray(Wp[idx, :]).astype(
            ml_dtypes.bfloat16).reshape(4, P, D)
        in_maps.append({"xb": xb16[b], "wq": wq_c, "wkv": wkv_c,
                        "wp": wp_c})
    res = run_bass_kernel_spmd(nc, in_maps, list(range(8)))
    outp = np.empty((B, N, D), np.float32)
    for b in range(B):
        acc = np.zeros((N, D), np.float32)
        for hp in range(4):
            acc += np.asarray(res.results[b * 4 + hp]["out"],
                              dtype=np.float32)
        outp[b] = acc + bp
    return outp
